# revision 1
# baseline (speedup 1.0000x reference)
"""CRF forward (log partition) on 8 NeuronCores, data-parallel over batch.

Math: the forward recurrence is run in probability space instead of log
space.  With E = exp(transitions) and G_t = exp(emissions_t), the CRF
recurrence alpha_{t+1} = logit_t + LSE_j(T + alpha_t) becomes the linear
recurrence P_{t+1} = G_t o (E @ P_t), with per-column renormalization every
few steps to stay in fp32 range (the log of each column's normalizer is
accumulated host-side from the stored reciprocals).

Variable lengths are handled by rewriting emissions: for t >= len[b] the
emission row is onehot(STOP), which makes the chain absorb into STOP (the
absorbing step computes exactly LSE_j(T[STOP,j] + alpha_j), i.e. the final
answer) and then drift by T[STOP,STOP] per extra step, which is corrected
exactly on the host: norm += (len - S) * T[STOP,STOP].

Per-core layout: 128 sequences are packed as 2 groups x 64 columns; the
state tile is [90, 64] (rows = 2 stacked copies of the 45 labels) and the
transition matmul uses blockdiag(E^T, E^T) as the stationary lhsT, so one
PE matmul advances all 128 sequences.
"""

import numpy as np

import concourse.bacc as bacc
import concourse.bass as bass
import concourse.mybir as mybir
import concourse.tile as tile
from concourse.bass_utils import run_bass_kernel_spmd

L = 45
START = 43
STOP = 44
B = 1024
S = 512
NCORES = 8
BPC = B // NCORES          # 128 sequences per core
NG = 2                     # groups per core
WCOL = BPC // NG           # 64 columns per group
PR = NG * L                # 90 partition rows for packed state
TSTEPS = S + 1             # +1 appended absorb step
RENORM = 6
NCHUNK = 9
CHUNK = TSTEPS // NCHUNK   # 57 steps per G chunk
assert CHUNK * NCHUNK == TSTEPS

F32 = mybir.dt.float32

_EVENTS = [t for t in range(TSTEPS) if (t + 1) % RENORM == 0 or t == TSTEPS - 1]
NEV = len(_EVENTS)


def _build_nc():
    # Bacc (not raw Bass): its legalization splits multi-sem waits into
    # standalone event-semaphore instructions, which walrus codegen requires.
    nc = bacc.Bacc("TRN2", target_bir_lowering=False, debug=False, num_devices=NCORES)
    g_dram = nc.dram_tensor("g", [PR, TSTEPS * WCOL], F32, kind="ExternalInput")
    e2t_dram = nc.dram_tensor("e2t", [PR, PR], F32, kind="ExternalInput")
    onesg_dram = nc.dram_tensor("onesg", [PR, NG], F32, kind="ExternalInput")
    indic_dram = nc.dram_tensor("indic", [NG, PR], F32, kind="ExternalInput")
    wout_dram = nc.dram_tensor("wout", [PR, WCOL], F32, kind="ExternalOutput")
    rstrip_dram = nc.dram_tensor("rstrip", [NG, NEV * WCOL], F32, kind="ExternalOutput")

    with tile.TileContext(nc) as tc:
        with (
            tc.tile_pool(name="const", bufs=1) as const_pool,
            tc.tile_pool(name="gchunks", bufs=NCHUNK) as g_pool,
            tc.tile_pool(name="state", bufs=3) as state_pool,
            tc.tile_pool(name="strip", bufs=1) as strip_pool,
            tc.tile_pool(name="ps_s", bufs=3, space="PSUM") as ps_s,
            tc.tile_pool(name="ps_n", bufs=2, space="PSUM") as ps_n,
            tc.tile_pool(name="ps_bc", bufs=2, space="PSUM") as ps_bc,
        ):
            # Matmult instructions encode only a few sem waits; DMAs can fan
            # out over many HWDGE queues (many sems).  Stage every matmul
            # input through a DVE copy so matmuls wait on compute sems only.
            e2t_st = const_pool.tile([PR, PR], F32, tag="e2t_st")
            nc.sync.dma_start(e2t_st[:], e2t_dram[:])
            e2t = const_pool.tile([PR, PR], F32, tag="e2t")
            nc.vector.tensor_copy(e2t[:], e2t_st[:])
            onesg_st = const_pool.tile([PR, NG], F32, tag="onesg_st")
            nc.sync.dma_start(onesg_st[:], onesg_dram[:])
            onesg = const_pool.tile([PR, NG], F32, tag="onesg")
            nc.vector.tensor_copy(onesg[:], onesg_st[:])
            indic_st = const_pool.tile([NG, PR], F32, tag="indic_st")
            nc.sync.dma_start(indic_st[:], indic_dram[:])
            indic = const_pool.tile([NG, PR], F32, tag="indic")
            nc.vector.tensor_copy(indic[:], indic_st[:])

            gtiles = []
            for c in range(NCHUNK):
                gt = g_pool.tile([PR, CHUNK * WCOL], F32, tag="g")
                nc.sync.dma_start(
                    gt[:], g_dram[:, c * CHUNK * WCOL : (c + 1) * CHUNK * WCOL]
                )
                gtiles.append(gt)

            rstrip = strip_pool.tile([NG, NEV * WCOL], F32, tag="rstrip")

            # Step 0 is folded host-side: the t=0 block of g already holds
            # W_0 = G_0 * E[:, START], the unnormalized state after step 0.
            pcur = state_pool.tile([PR, WCOL], F32, tag="w")
            nc.vector.tensor_copy(pcur[:], gtiles[0][:, 0:WCOL])

            ev = 0
            for t in range(1, TSTEPS):
                s_ps = ps_s.tile([PR, WCOL], F32, tag="s")
                nc.tensor.matmul(s_ps[:], e2t[:], pcur[:], start=True, stop=True)
                gslice = gtiles[t // CHUNK][
                    :, (t % CHUNK) * WCOL : (t % CHUNK + 1) * WCOL
                ]
                w = state_pool.tile([PR, WCOL], F32, tag="w")
                nc.vector.tensor_mul(w[:], gslice, s_ps[:])
                if (t + 1) % RENORM == 0 or t == TSTEPS - 1:
                    n_ps = ps_n.tile([NG, WCOL], F32, tag="n")
                    nc.tensor.matmul(n_ps[:], onesg[:], w[:], start=True, stop=True)
                    rslot = rstrip[:, ev * WCOL : (ev + 1) * WCOL]
                    nc.vector.reciprocal(rslot, n_ps[:])
                    bc_ps = ps_bc.tile([PR, WCOL], F32, tag="bc")
                    nc.tensor.matmul(bc_ps[:], indic[:], rslot, start=True, stop=True)
                    pnew = state_pool.tile([PR, WCOL], F32, tag="w")
                    nc.vector.tensor_mul(pnew[:], w[:], bc_ps[:])
                    pcur = pnew
                    ev += 1
                else:
                    pcur = w

            assert ev == NEV
            nc.sync.dma_start(wout_dram[:], pcur[:])
            nc.sync.dma_start(rstrip_dram[:], rstrip[:])

    nc.compile()
    return nc


_NC_CACHE = {}


def _get_nc():
    if "nc" not in _NC_CACHE:
        _NC_CACHE["nc"] = _build_nc()
    return _NC_CACHE["nc"]


def _prep_inputs(logits, lens, transitions):
    """Host-side preprocessing: exp + absorb-rewrite + per-core packing."""
    logits = np.asarray(logits, np.float32)
    lens = np.asarray(lens, np.int32)
    T = np.asarray(transitions, np.float32)

    E = np.exp(T.astype(np.float32))
    e2t = np.zeros((PR, PR), np.float32)
    e2t[:L, :L] = E.T
    e2t[L:, L:] = E.T

    onesg = np.zeros((PR, NG), np.float32)
    onesg[:L, 0] = 1.0
    onesg[L:, 1] = 1.0
    indic = np.ascontiguousarray(onesg.T)

    G = np.exp(logits)
    absorbed = np.arange(S)[None, :, None] >= lens[:, None, None]
    onehot = np.zeros(L, np.float32)
    onehot[STOP] = 1.0
    G = np.where(absorbed, onehot[None, None, :], G)
    G = np.concatenate(
        [G, np.broadcast_to(onehot, (B, 1, L)).astype(np.float32)], axis=1
    )  # [B, TSTEPS, L]

    # Fold step 0: the t=0 block becomes W_0 = G_0 * E[:, START].
    G[:, 0, :] *= E[:, START][None, :]

    in_maps = []
    for c in range(NCORES):
        gc = G[c * BPC : (c + 1) * BPC].reshape(NG, WCOL, TSTEPS, L)
        g_in = np.ascontiguousarray(
            np.transpose(gc, (0, 3, 2, 1)).reshape(PR, TSTEPS * WCOL)
        )
        in_maps.append({"g": g_in, "e2t": e2t, "onesg": onesg, "indic": indic})
    return in_maps


def _postprocess(results, lens, transitions):
    lens = np.asarray(lens, np.int64)
    T = np.asarray(transitions, np.float32)
    t_ss = np.float64(T[STOP, STOP])
    norm = np.empty(B, np.float64)
    for c in range(NCORES):
        wout = np.asarray(results[c]["wout"])      # [PR, WCOL]
        rstrip = np.asarray(results[c]["rstrip"])  # [NG, NEV*WCOL]
        rs = rstrip.reshape(NG, NEV, WCOL).astype(np.float64)
        z = -np.log(rs).sum(axis=1)                # [NG, WCOL]
        for g in range(NG):
            pstop = wout[g * L + STOP, :].astype(np.float64)
            sl = slice(c * BPC + g * WCOL, c * BPC + (g + 1) * WCOL)
            norm[sl] = np.log(pstop) + z[g] + (lens[sl] - S) * t_ss
    return norm.astype(np.float32)


def kernel(logits, lens, transitions):
    nc = _get_nc()
    in_maps = _prep_inputs(logits, lens, transitions)
    res = run_bass_kernel_spmd(nc, in_maps, list(range(NCORES)))
    return _postprocess(res.results, lens, transitions)



# revision 5
# speedup vs baseline: 2.2009x; 2.2009x over previous
"""CRF forward (log partition) on 8 NeuronCores, data-parallel over batch.

Math: the forward recurrence runs in probability space: with E = exp(T) and
G_t = exp(emissions_t), alpha_{t+1} = logit_t + LSE_j(T + alpha_t) becomes the
linear recurrence P_{t+1} = G_t o (E @ P_t).

All normalization is folded into the DATA on the host: each active step's
emission row is pre-scaled by 1/m_t[b] with m_t[b] = sum_i G[b,t,i]*rowmean(E)_i
(a deterministic per-sequence scalar), which keeps the state O(1) in bf16 range
without any data-dependent renorm on device.  The log-scales are accumulated in
float64 host-side and added back at the end.

Variable lengths via an extra DONE label D per group (46 labels on device):
E'[D,:45] = E[STOP,:], E'[D,D] = 1.0 (exact in bf16), column D otherwise 0.
Active steps emit 0 for D so P[D] stays exactly 0; the absorb step at t=len[b]
emits onehot(D), capturing LSE_j(T[STOP,j]+alpha_j) -- the final answer -- into
P[D]; all later steps emit onehot(D) again, multiplying P[D] by exactly 1.0
(zero bf16 drift) and keeping the other labels exactly 0.

Per-core layout: 128 sequences packed as 2 groups x 64 columns; state tile
[92, 64] (2 stacked copies of the 46 labels); the transition matmul uses
blockdiag(E'^T, E'^T) as stationary lhsT, loaded into the PE array ONCE via a
standalone ldweights; every step matmul is marked non-self-loading
(ldweights=False), so the per-step critical path is just
matmul (PE) -> tensor_mul (DVE) in bf16.
"""

import numpy as np
import ml_dtypes

import concourse.bacc as bacc
import concourse.bass as bass
import concourse.mybir as mybir
import concourse.tile as tile
from concourse.bass_utils import run_bass_kernel_spmd

L = 45
START = 43
STOP = 44
LD = 46                    # labels + DONE landing pad
DONE = 45
B = 1024
S = 512
NCORES = 8
BPC = B // NCORES          # 128 sequences per core
NG = 2                     # groups per core
WCOL = BPC // NG           # 64 columns per group
PR = NG * LD               # 92 partition rows for packed state
TSTEPS = S + 1             # +1 appended absorb step
NCHUNK = 9
CHUNK = TSTEPS // NCHUNK   # 57 steps per G chunk
assert CHUNK * NCHUNK == TSTEPS

F32 = mybir.dt.float32
BF16 = mybir.dt.bfloat16
NP_BF16 = ml_dtypes.bfloat16


def _build_nc():
    # Bacc (not raw Bass): its legalization splits multi-sem waits into
    # standalone event-semaphore instructions, which walrus codegen requires.
    nc = bacc.Bacc("TRN2", target_bir_lowering=False, debug=False, num_devices=NCORES)
    g_dram = nc.dram_tensor("g", [PR, TSTEPS * WCOL], BF16, kind="ExternalInput")
    e2t_dram = nc.dram_tensor("e2t", [PR, PR], BF16, kind="ExternalInput")
    wout_dram = nc.dram_tensor("wout", [PR, WCOL], BF16, kind="ExternalOutput")

    with tile.TileContext(nc) as tc:
        with (
            tc.tile_pool(name="const", bufs=1) as const_pool,
            tc.tile_pool(name="gchunks", bufs=NCHUNK) as g_pool,
            tc.tile_pool(name="state", bufs=3) as state_pool,
            tc.tile_pool(name="ps_s", bufs=3, space="PSUM") as ps_s,
        ):
            # Stage the stationary through a DVE copy so the ldweights waits
            # on a compute sem only (matmult-encodable), not a DMA sem.
            e2t_st = const_pool.tile([PR, PR], BF16, tag="e2t_st")
            nc.sync.dma_start(e2t_st[:], e2t_dram[:])
            e2t = const_pool.tile([PR, PR], BF16, tag="e2t")
            nc.vector.tensor_copy(e2t[:], e2t_st[:])

            gtiles = []
            for c in range(NCHUNK):
                gt = g_pool.tile([PR, CHUNK * WCOL], BF16, tag="g")
                nc.sync.dma_start(
                    gt[:], g_dram[:, c * CHUNK * WCOL : (c + 1) * CHUNK * WCOL]
                )
                gtiles.append(gt)

            # Load blockdiag(E'^T, E'^T) into the PE array once; every step
            # matmul below reuses it (ldweights=False).
            nc.tensor.ldweights(e2t[:])

            # Step 0 is folded host-side: the t=0 block of g already holds
            # W_0 = normalize(G_0 * E[:, START]).
            pcur = gtiles[0][:, 0:WCOL]

            for t in range(1, TSTEPS):
                s_ps = ps_s.tile([PR, WCOL], F32, tag="s")
                mm = nc.tensor.matmul(s_ps[:], e2t[:], pcur, start=True, stop=True)
                mm.ins.ldweights = False
                gslice = gtiles[t // CHUNK][
                    :, (t % CHUNK) * WCOL : (t % CHUNK + 1) * WCOL
                ]
                w = state_pool.tile([PR, WCOL], BF16, tag="w")
                nc.vector.tensor_mul(w[:], gslice, s_ps[:])
                pcur = w[:]

            nc.sync.dma_start(wout_dram[:], pcur)

    nc.compile()
    return nc


_NC_CACHE = {}


def _get_nc():
    if "nc" not in _NC_CACHE:
        _NC_CACHE["nc"] = _build_nc()
    return _NC_CACHE["nc"]


def _prep_inputs(logits, lens, transitions):
    """Host-side preprocessing: exp + absorb-rewrite + deterministic
    per-(seq,step) scaling + per-core packing.  Stashes the float64
    log-scale accumulator for _postprocess."""
    logits = np.asarray(logits, np.float32)
    lens = np.asarray(lens, np.int64)
    T = np.asarray(transitions, np.float64)

    E = np.exp(T)                      # [45,45] float64
    erow = E.mean(axis=1)              # mean_j E[i,j], [45]

    Eg = np.zeros((LD, LD), np.float64)
    Eg[:L, :L] = E
    Eg[DONE, :L] = E[STOP, :]
    Eg[DONE, DONE] = 1.0
    e2t = np.zeros((PR, PR), np.float64)
    e2t[:LD, :LD] = Eg.T
    e2t[LD:, LD:] = Eg.T

    G = np.exp(logits.astype(np.float64))          # [B,S,45]

    t_idx = np.arange(S)[None, :]                  # [1,S]
    active = t_idx < lens[:, None]                 # [B,S]

    # Fold step 0 and normalize it exactly: W0 = G0*E[:,START], scale 1/sum.
    W0 = G[:, 0, :] * E[:, START][None, :]         # [B,45]
    m0 = W0.sum(axis=1)                            # [B]
    G[:, 0, :] = W0 / m0[:, None]

    # Active steps t>=1: scale by 1/m_t, m_t = sum_i G_t[i]*erow[i].
    m = G @ erow                                   # [B,S]
    scale_mask = active & (t_idx > 0)
    np.divide(G, m[:, :, None], out=G, where=scale_mask[:, :, None])

    # log-scale accumulator: z[b] = log m0 + sum_{1<=t<len} log m_t.
    logm = np.where(scale_mask, np.log(m), 0.0)
    z = np.log(m0) + logm.sum(axis=1)

    # 46-label emissions: D gets 0 while active, onehot(D) from t>=len on.
    G46 = np.zeros((B, TSTEPS, LD), np.float64)
    G46[:, :S, :L] = np.where(active[:, :, None], G, 0.0)
    done_from = t_idx >= lens[:, None]             # includes absorb step
    G46[:, :S, DONE] = np.where(done_from, 1.0, 0.0)
    G46[:, S, DONE] = 1.0                          # appended step

    _NC_CACHE["z"] = z

    g16 = G46.astype(NP_BF16)
    e2t16 = e2t.astype(NP_BF16)
    in_maps = []
    for c in range(NCORES):
        gc = g16[c * BPC : (c + 1) * BPC].reshape(NG, WCOL, TSTEPS, LD)
        g_in = np.ascontiguousarray(
            np.transpose(gc, (0, 3, 2, 1)).reshape(PR, TSTEPS * WCOL)
        )
        in_maps.append({"g": g_in, "e2t": e2t16})
    return in_maps


def _postprocess(results, lens, transitions):
    z = _NC_CACHE["z"]
    norm = np.empty(B, np.float64)
    for c in range(NCORES):
        wout = np.asarray(results[c]["wout"]).astype(np.float64)  # [PR, WCOL]
        for g in range(NG):
            pdone = wout[g * LD + DONE, :]
            sl = slice(c * BPC + g * WCOL, c * BPC + (g + 1) * WCOL)
            norm[sl] = np.log(pdone) + z[sl]
    return norm.astype(np.float32)


def kernel(logits, lens, transitions):
    nc = _get_nc()
    in_maps = _prep_inputs(logits, lens, transitions)
    res = run_bass_kernel_spmd(nc, in_maps, list(range(NCORES)))
    return _postprocess(res.results, lens, transitions)


# revision 8
# speedup vs baseline: 2.4377x; 1.1076x over previous
"""CRF forward (log partition) on 8 NeuronCores, data-parallel over batch.

Math: the forward recurrence runs in probability space: with E = exp(T) and
G_t = exp(emissions_t), alpha_{t+1} = logit_t + LSE_j(T + alpha_t) becomes the
linear recurrence P_{t+1} = G_t o (E @ P_t).

All normalization is folded into the DATA on the host: each active step's
emission row is pre-scaled by 1/m_t[b] with m_t[b] = sum_i G[b,t,i]*rowmean(E)_i
(a deterministic per-sequence scalar), which keeps the state O(1) in bf16 range
without any data-dependent renorm on device.  The log-scales are accumulated in
float64 host-side and added back at the end.

Variable lengths via an extra DONE label D per group (46 labels on device):
E'[D,:45] = E[STOP,:], E'[D,D] = 1.0 (exact in bf16), column D otherwise 0.
Active steps emit 0 for D so P[D] stays exactly 0; the absorb step at t=len[b]
emits onehot(D), capturing LSE_j(T[STOP,j]+alpha_j) -- the final answer -- into
P[D]; later steps emit onehot(D) again, multiplying P[D] by exactly 1.0.

Shrinking-width steps: sequences are dealt longest-first round-robin across the
16 (core, group) slots, and within each slot sorted descending into columns, so
column k's sequence dies no later than a STATIC schedule width n_t allows.  The
state lives in ONE in-place tile; step t only updates columns [0, n_t), so dead
columns keep their DONE value frozen.  Any sequence too long for its column
(impossible under the static margin for uniform lengths, but checked) is
computed exactly on the host instead.

Per-core critical path per step: one bf16 matmul [92,92]x[92,n_t] with the
stationary blockdiag(E'^T,E'^T) kept loaded in the PE array (standalone
ldweights + non-self-loading matmuls), then one DVE tensor_mul.
"""

import numpy as np
import ml_dtypes

import concourse.bacc as bacc
import concourse.bass as bass
import concourse.mybir as mybir
import concourse.tile as tile
from concourse.bass_utils import run_bass_kernel_spmd

L = 45
START = 43
STOP = 44
LD = 46                    # labels + DONE landing pad
DONE = 45
B = 1024
S = 512
NCORES = 8
BPC = B // NCORES          # 128 sequences per core
NG = 2                     # groups per core
WCOL = BPC // NG           # 64 columns per group
PR = NG * LD               # 92 partition rows for packed state
TSTEPS = S + 1             # +1 appended absorb step
NSLOTS = NCORES * NG       # 16 (core, group) slots

F32 = mybir.dt.float32
BF16 = mybir.dt.bfloat16
NP_BF16 = ml_dtypes.bfloat16

# Static shrinking-width schedule: step t in [1, 512] updates columns [0, n_t).
_T_ARR = np.arange(1, TSTEPS)
_N_SCHED = np.minimum(
    WCOL, np.maximum(8, np.ceil(WCOL * (TSTEPS - _T_ARR) / TSTEPS).astype(int) + 3)
)
# Column lifetime: last step that still updates column k.
_T_COL = np.array(
    [int((np.where(_N_SCHED > k)[0] + 1).max()) for k in range(WCOL)], np.int64
)
# Per-step g block widths (block 0 is the full-width init state) and offsets.
_BLK_W = np.concatenate([[WCOL], _N_SCHED])          # [TSTEPS]
_BLK_OFF = np.concatenate([[0], np.cumsum(_BLK_W)])  # [TSTEPS+1]
GCOLS = int(_BLK_OFF[-1])
# Chunk boundaries (step indices): small first chunk for a fast pipeline start.
_CHUNK_STEPS = [0, 8] + list(np.linspace(8, TSTEPS, 9).astype(int)[1:])
NCHUNK = len(_CHUNK_STEPS) - 1


def _build_nc():
    # Bacc (not raw Bass): its legalization splits multi-sem waits into
    # standalone event-semaphore instructions, which walrus codegen requires.
    nc = bacc.Bacc("TRN2", target_bir_lowering=False, debug=False, num_devices=NCORES)
    g_dram = nc.dram_tensor("g", [PR, GCOLS], BF16, kind="ExternalInput")
    e2t_dram = nc.dram_tensor("e2t", [PR, PR], BF16, kind="ExternalInput")
    wout_dram = nc.dram_tensor("wout", [PR, WCOL], BF16, kind="ExternalOutput")

    with tile.TileContext(nc) as tc:
        with (
            tc.tile_pool(name="const", bufs=1) as const_pool,
            tc.tile_pool(name="gchunks", bufs=1) as g_pool,
            tc.tile_pool(name="state", bufs=1) as state_pool,
            tc.tile_pool(name="ps_s", bufs=3, space="PSUM") as ps_s,
        ):
            # Stage the stationary through a DVE copy so the ldweights waits
            # on a compute sem only (matmult-encodable), not a DMA sem.
            e2t_st = const_pool.tile([PR, PR], BF16, tag="e2t_st")
            nc.sync.dma_start(e2t_st[:], e2t_dram[:])
            e2t = const_pool.tile([PR, PR], BF16, tag="e2t")
            nc.vector.tensor_copy(e2t[:], e2t_st[:])

            gtiles = []
            for c in range(NCHUNK):
                c0 = int(_BLK_OFF[_CHUNK_STEPS[c]])
                c1 = int(_BLK_OFF[_CHUNK_STEPS[c + 1]])
                gt = g_pool.tile([PR, c1 - c0], BF16, tag=f"g{c}")
                nc.sync.dma_start(gt[:], g_dram[:, c0:c1])
                gtiles.append(gt)

            # Load blockdiag(E'^T, E'^T) into the PE array once; every step
            # matmul below reuses it (ldweights=False).
            nc.tensor.ldweights(e2t[:])

            # In-place state; initialized from the host-folded W_0 block.
            w_state = state_pool.tile([PR, WCOL], BF16, tag="w")
            nc.vector.tensor_copy(w_state[:], gtiles[0][:, 0:WCOL])

            chunk_of = np.searchsorted(_CHUNK_STEPS, np.arange(TSTEPS), "right") - 1
            for t in range(1, TSTEPS):
                n = int(_N_SCHED[t - 1])
                s_ps = ps_s.tile([PR, WCOL], F32, tag="s")
                mm = nc.tensor.matmul(
                    s_ps[:, 0:n], e2t[:], w_state[:, 0:n], start=True, stop=True
                )
                mm.ins.ldweights = False
                c = int(chunk_of[t])
                off = int(_BLK_OFF[t] - _BLK_OFF[_CHUNK_STEPS[c]])
                nc.vector.tensor_mul(
                    w_state[:, 0:n], gtiles[c][:, off : off + n], s_ps[:, 0:n]
                )

            nc.sync.dma_start(wout_dram[:], w_state[:])

    nc.compile()
    return nc


_NC_CACHE = {}


def _get_nc():
    if "nc" not in _NC_CACHE:
        _NC_CACHE["nc"] = _build_nc()
    return _NC_CACHE["nc"]


def _host_norm(logit_b, len_b, T):
    """Exact float64 log-space forward for one sequence (fallback path)."""
    NEG_INF = -10000.0
    alpha = np.full(L, NEG_INF)
    alpha[START] = 0.0
    for t in range(len_b):
        mat = T + alpha[None, :]
        mx = mat.max(axis=1)
        alpha = logit_b[t] + np.log(np.exp(mat - mx[:, None]).sum(axis=1)) + mx
    v = alpha + T[STOP]
    mx = v.max()
    return np.log(np.exp(v - mx).sum()) + mx


def _prep_inputs(logits, lens, transitions):
    """Host-side preprocessing: exp + absorb-rewrite + deterministic
    per-(seq,step) scaling + length-sorted packing.  Stashes the float64
    log-scale accumulator, the column permutation, and any host-fallback
    results for _postprocess."""
    logits = np.asarray(logits, np.float32)
    lens = np.asarray(lens, np.int64)
    T = np.asarray(transitions, np.float64)

    E = np.exp(T)                      # [45,45] float64
    erow = E.mean(axis=1)              # mean_j E[i,j], [45]

    Eg = np.zeros((LD, LD), np.float64)
    Eg[:L, :L] = E
    Eg[DONE, :L] = E[STOP, :]
    Eg[DONE, DONE] = 1.0
    e2t = np.zeros((PR, PR), np.float64)
    e2t[:LD, :LD] = Eg.T
    e2t[LD:, LD:] = Eg.T

    G = np.exp(logits.astype(np.float64))          # [B,S,45]

    t_idx = np.arange(S)[None, :]                  # [1,S]
    active = t_idx < lens[:, None]                 # [B,S]

    # Fold step 0 and normalize it exactly: W0 = G0*E[:,START], scale 1/sum.
    W0 = G[:, 0, :] * E[:, START][None, :]         # [B,45]
    m0 = W0.sum(axis=1)                            # [B]
    G[:, 0, :] = W0 / m0[:, None]

    # Active steps t>=1: scale by 1/m_t, m_t = sum_i G_t[i]*erow[i].
    m = G @ erow                                   # [B,S]
    scale_mask = active & (t_idx > 0)
    np.divide(G, m[:, :, None], out=G, where=scale_mask[:, :, None])

    # log-scale accumulator: z[b] = log m0 + sum_{1<=t<len} log m_t.
    logm = np.where(scale_mask, np.log(m), 0.0)
    z = np.log(m0) + logm.sum(axis=1)

    # 46-label emissions: D gets 0 while active, onehot(D) from t>=len on.
    G46 = np.zeros((B, TSTEPS, LD), np.float64)
    G46[:, :S, :L] = np.where(active[:, :, None], G, 0.0)
    done_from = t_idx >= lens[:, None]             # includes absorb step
    G46[:, :S, DONE] = np.where(done_from, 1.0, 0.0)
    G46[:, S, DONE] = 1.0                          # appended step

    # Deal longest-first round-robin across the 16 (core, group) slots.
    order = np.argsort(-lens, kind="stable")
    slots = np.empty((NSLOTS, WCOL), np.int64)
    for r, b in enumerate(order):
        slots[r % NSLOTS][r // NSLOTS] = b
    # Host fallback for any sequence outliving its column's static lifetime.
    host_norms = {}
    logits64 = logits.astype(np.float64)
    for s in range(NSLOTS):
        for k in range(WCOL):
            b = slots[s][k]
            if lens[b] > _T_COL[k]:
                host_norms[int(b)] = _host_norm(logits64[b], int(lens[b]), T)

    _NC_CACHE["z"] = z
    _NC_CACHE["slots"] = slots
    _NC_CACHE["host_norms"] = host_norms

    g16 = G46.astype(NP_BF16)
    e2t16 = e2t.astype(NP_BF16)
    in_maps = []
    for c in range(NCORES):
        g_in = np.zeros((PR, GCOLS), NP_BF16)
        for g in range(NG):
            seqs = slots[c * NG + g]               # [WCOL] original indices
            rows = slice(g * LD, (g + 1) * LD)
            # Per-step blocks: step t occupies cols [_BLK_OFF[t], +width).
            gc = g16[seqs]                         # [WCOL, TSTEPS, LD]
            for t in range(TSTEPS):
                w = int(_BLK_W[t])
                o = int(_BLK_OFF[t])
                g_in[rows, o : o + w] = gc[:w, t, :].T
        in_maps.append({"g": g_in, "e2t": e2t16})
    return in_maps


def _postprocess(results, lens, transitions):
    z = _NC_CACHE["z"]
    slots = _NC_CACHE["slots"]
    host_norms = _NC_CACHE["host_norms"]
    norm = np.empty(B, np.float64)
    for c in range(NCORES):
        wout = np.asarray(results[c]["wout"]).astype(np.float64)  # [PR, WCOL]
        for g in range(NG):
            seqs = slots[c * NG + g]
            pdone = wout[g * LD + DONE, :]
            norm[seqs] = np.log(pdone) + z[seqs]
    for b, v in host_norms.items():
        norm[b] = v
    return norm.astype(np.float32)


def kernel(logits, lens, transitions):
    nc = _get_nc()
    in_maps = _prep_inputs(logits, lens, transitions)
    res = run_bass_kernel_spmd(nc, in_maps, list(range(NCORES)))
    return _postprocess(res.results, lens, transitions)


# revision 10
# speedup vs baseline: 2.4728x; 1.0144x over previous
"""CRF forward (log partition) on 8 NeuronCores, data-parallel over batch.

Math: the forward recurrence runs in probability space: with E = exp(T) and
G_t = exp(emissions_t), alpha_{t+1} = logit_t + LSE_j(T + alpha_t) becomes the
linear recurrence P_{t+1} = G_t o (E @ P_t).

All normalization is folded into the DATA on the host: each active step's
emission row is pre-scaled by 1/m_t[b] with m_t[b] = sum_i G[b,t,i]*rowmean(E)_i
(a deterministic per-sequence scalar), which keeps the state O(1) in bf16 range
without any data-dependent renorm on device.  The log-scales are accumulated in
float64 host-side and added back at the end.

Variable lengths via an extra DONE label D per group (46 labels on device):
E'[D,:45] = E[STOP,:], E'[D,D] = 1.0 (exact in bf16), column D otherwise 0.
Active steps emit 0 for D so P[D] stays exactly 0; the absorb step at t=len[b]
emits onehot(D), capturing LSE_j(T[STOP,j]+alpha_j) -- the final answer -- into
P[D]; later steps emit onehot(D) again, multiplying P[D] by exactly 1.0.

Shrinking-width steps: sequences are dealt longest-first round-robin across the
16 (core, group) slots, and within each slot sorted descending into columns, so
column k's sequence dies no later than a STATIC schedule width n_t allows.  The
state lives in ONE in-place tile; step t only updates columns [0, n_t), so dead
columns keep their DONE value frozen.  Any sequence too long for its column
(impossible under the static margin for uniform lengths, but checked) is
computed exactly on the host instead.

Per-core critical path per step: one bf16 matmul [92,92]x[92,n_t] with the
stationary blockdiag(E'^T,E'^T) kept loaded in the PE array (standalone
ldweights + non-self-loading matmuls), then one DVE tensor_mul.
"""

import numpy as np
import ml_dtypes

import concourse.bacc as bacc
import concourse.bass as bass
import concourse.mybir as mybir
import concourse.tile as tile
from concourse.bass_utils import run_bass_kernel_spmd

L = 45
START = 43
STOP = 44
LD = 46                    # labels + DONE landing pad
DONE = 45
B = 1024
S = 512
NCORES = 8
BPC = B // NCORES          # 128 sequences per core
NG = 2                     # groups per core
WCOL = BPC // NG           # 64 columns per group
PR = NG * LD               # 92 partition rows for packed state
TSTEPS = S + 1             # +1 appended absorb step
NSLOTS = NCORES * NG       # 16 (core, group) slots

F32 = mybir.dt.float32
BF16 = mybir.dt.bfloat16
NP_BF16 = ml_dtypes.bfloat16

# Static shrinking-width schedule: step t in [1, 512] updates columns [0, n_t).
# n_1 == WCOL always, so the in-place state is fully written by step 1.
_T_ARR = np.arange(1, TSTEPS)
_N_SCHED = np.minimum(
    WCOL, np.maximum(4, np.ceil(WCOL * (TSTEPS - _T_ARR) / TSTEPS).astype(int) + 2)
)
assert _N_SCHED[0] == WCOL
# Column lifetime: last step that still updates column k.
_T_COL = np.array(
    [int((np.where(_N_SCHED > k)[0] + 1).max()) for k in range(WCOL)], np.int64
)
# Per-step g block widths (block 0 is the full-width init state) and offsets.
_BLK_W = np.concatenate([[WCOL], _N_SCHED])          # [TSTEPS]
_BLK_OFF = np.concatenate([[0], np.cumsum(_BLK_W)])  # [TSTEPS+1]
GCOLS = int(_BLK_OFF[-1])
# Chunk boundaries (step indices): tiny leading chunks for a fast start.
_CHUNK_STEPS = [0, 1, 9, 41] + list(np.linspace(41, TSTEPS, 7).astype(int)[1:])
NCHUNK = len(_CHUNK_STEPS) - 1


def _build_nc():
    # Bacc (not raw Bass): its legalization splits multi-sem waits into
    # standalone event-semaphore instructions, which walrus codegen requires.
    nc = bacc.Bacc("TRN2", target_bir_lowering=False, debug=False, num_devices=NCORES)
    g_dram = nc.dram_tensor("g", [PR, GCOLS], BF16, kind="ExternalInput")
    e2t_dram = nc.dram_tensor("e2t", [PR, PR], BF16, kind="ExternalInput")
    wout_dram = nc.dram_tensor("wout", [PR, WCOL], BF16, kind="ExternalOutput")

    with tile.TileContext(nc) as tc:
        with (
            tc.tile_pool(name="const", bufs=1) as const_pool,
            tc.tile_pool(name="gchunks", bufs=1) as g_pool,
            tc.tile_pool(name="state", bufs=1) as state_pool,
            tc.tile_pool(name="ps_s", bufs=3, space="PSUM") as ps_s,
        ):
            e2t = const_pool.tile([PR, PR], BF16, tag="e2t")
            nc.sync.dma_start(e2t[:], e2t_dram[:])

            gtiles = []
            for c in range(NCHUNK):
                c0 = int(_BLK_OFF[_CHUNK_STEPS[c]])
                c1 = int(_BLK_OFF[_CHUNK_STEPS[c + 1]])
                gt = g_pool.tile([PR, c1 - c0], BF16, tag=f"g{c}")
                nc.sync.dma_start(gt[:], g_dram[:, c0:c1])
                gtiles.append(gt)

            # Load blockdiag(E'^T, E'^T) into the PE array once; every step
            # matmul below reuses it (redundant auto-ldweights are stripped
            # after tile legalization below).
            nc.tensor.ldweights(e2t[:])

            # In-place state: step 1 is full width, so the state tile is
            # fully written by the first tensor_mul; the first matmul's
            # moving operand is the host-folded W_0 block of g directly.
            w_state = state_pool.tile([PR, WCOL], BF16, tag="w")

            chunk_of = np.searchsorted(_CHUNK_STEPS, np.arange(TSTEPS), "right") - 1
            for t in range(1, TSTEPS):
                n = int(_N_SCHED[t - 1])
                moving = gtiles[0][:, 0:WCOL] if t == 1 else w_state[:, 0:n]
                s_ps = ps_s.tile([PR, WCOL], F32, tag="s")
                nc.tensor.matmul(s_ps[:, 0:n], e2t[:], moving, start=True, stop=True)
                c = int(chunk_of[t])
                off = int(_BLK_OFF[t] - _BLK_OFF[_CHUNK_STEPS[c]])
                nc.vector.tensor_mul(
                    w_state[:, 0:n], gtiles[c][:, off : off + n], s_ps[:, 0:n]
                )

            nc.sync.dma_start(wout_dram[:], w_state[:])

    # Tile legalization splits every bf16 matmult into LDWEIGHTS + MATMULT.
    # All those loads are of the SAME stationary tile, so keep only the
    # first (the explicit one above) and drop the rest.  The auto-inserted
    # loads carry no semaphore waits/updates (all sync lives on the
    # matmults), so removal is sync-neutral.
    kept_first = False
    for blk in nc.main_func.blocks:
        for i in list(blk.instructions):
            if isinstance(i, mybir.InstLdweights):
                if not kept_first:
                    kept_first = True
                elif i.sync_info is None:
                    blk.instructions.remove(i)

    nc.compile()
    return nc


_NC_CACHE = {}


def _get_nc():
    if "nc" not in _NC_CACHE:
        _NC_CACHE["nc"] = _build_nc()
    return _NC_CACHE["nc"]


def _host_norm(logit_b, len_b, T):
    """Exact float64 log-space forward for one sequence (fallback path)."""
    NEG_INF = -10000.0
    alpha = np.full(L, NEG_INF)
    alpha[START] = 0.0
    for t in range(len_b):
        mat = T + alpha[None, :]
        mx = mat.max(axis=1)
        alpha = logit_b[t] + np.log(np.exp(mat - mx[:, None]).sum(axis=1)) + mx
    v = alpha + T[STOP]
    mx = v.max()
    return np.log(np.exp(v - mx).sum()) + mx


def _prep_inputs(logits, lens, transitions):
    """Host-side preprocessing: exp + absorb-rewrite + deterministic
    per-(seq,step) scaling + length-sorted packing.  Stashes the float64
    log-scale accumulator, the column permutation, and any host-fallback
    results for _postprocess."""
    logits = np.asarray(logits, np.float32)
    lens = np.asarray(lens, np.int64)
    T = np.asarray(transitions, np.float64)

    E = np.exp(T)                      # [45,45] float64
    erow = E.mean(axis=1)              # mean_j E[i,j], [45]

    Eg = np.zeros((LD, LD), np.float64)
    Eg[:L, :L] = E
    Eg[DONE, :L] = E[STOP, :]
    Eg[DONE, DONE] = 1.0
    e2t = np.zeros((PR, PR), np.float64)
    e2t[:LD, :LD] = Eg.T
    e2t[LD:, LD:] = Eg.T

    G = np.exp(logits.astype(np.float64))          # [B,S,45]

    t_idx = np.arange(S)[None, :]                  # [1,S]
    active = t_idx < lens[:, None]                 # [B,S]

    # Fold step 0 and normalize it exactly: W0 = G0*E[:,START], scale 1/sum.
    W0 = G[:, 0, :] * E[:, START][None, :]         # [B,45]
    m0 = W0.sum(axis=1)                            # [B]
    G[:, 0, :] = W0 / m0[:, None]

    # Active steps t>=1: scale by 1/m_t, m_t = sum_i G_t[i]*erow[i].
    m = G @ erow                                   # [B,S]
    scale_mask = active & (t_idx > 0)
    np.divide(G, m[:, :, None], out=G, where=scale_mask[:, :, None])

    # log-scale accumulator: z[b] = log m0 + sum_{1<=t<len} log m_t.
    logm = np.where(scale_mask, np.log(m), 0.0)
    z = np.log(m0) + logm.sum(axis=1)

    # 46-label emissions: D gets 0 while active, onehot(D) from t>=len on.
    G46 = np.zeros((B, TSTEPS, LD), np.float64)
    G46[:, :S, :L] = np.where(active[:, :, None], G, 0.0)
    done_from = t_idx >= lens[:, None]             # includes absorb step
    G46[:, :S, DONE] = np.where(done_from, 1.0, 0.0)
    G46[:, S, DONE] = 1.0                          # appended step

    # Deal longest-first round-robin across the 16 (core, group) slots.
    order = np.argsort(-lens, kind="stable")
    slots = np.empty((NSLOTS, WCOL), np.int64)
    for r, b in enumerate(order):
        slots[r % NSLOTS][r // NSLOTS] = b
    # Host fallback for any sequence outliving its column's static lifetime.
    host_norms = {}
    logits64 = logits.astype(np.float64)
    for s in range(NSLOTS):
        for k in range(WCOL):
            b = slots[s][k]
            if lens[b] > _T_COL[k]:
                host_norms[int(b)] = _host_norm(logits64[b], int(lens[b]), T)

    _NC_CACHE["z"] = z
    _NC_CACHE["slots"] = slots
    _NC_CACHE["host_norms"] = host_norms

    g16 = G46.astype(NP_BF16)
    e2t16 = e2t.astype(NP_BF16)
    in_maps = []
    for c in range(NCORES):
        g_in = np.zeros((PR, GCOLS), NP_BF16)
        for g in range(NG):
            seqs = slots[c * NG + g]               # [WCOL] original indices
            rows = slice(g * LD, (g + 1) * LD)
            # Per-step blocks: step t occupies cols [_BLK_OFF[t], +width).
            gc = g16[seqs]                         # [WCOL, TSTEPS, LD]
            for t in range(TSTEPS):
                w = int(_BLK_W[t])
                o = int(_BLK_OFF[t])
                g_in[rows, o : o + w] = gc[:w, t, :].T
        in_maps.append({"g": g_in, "e2t": e2t16})
    return in_maps


def _postprocess(results, lens, transitions):
    z = _NC_CACHE["z"]
    slots = _NC_CACHE["slots"]
    host_norms = _NC_CACHE["host_norms"]
    norm = np.empty(B, np.float64)
    for c in range(NCORES):
        wout = np.asarray(results[c]["wout"]).astype(np.float64)  # [PR, WCOL]
        for g in range(NG):
            seqs = slots[c * NG + g]
            pdone = wout[g * LD + DONE, :]
            norm[seqs] = np.log(pdone) + z[seqs]
    for b, v in host_norms.items():
        norm[b] = v
    return norm.astype(np.float32)


def kernel(logits, lens, transitions):
    nc = _get_nc()
    in_maps = _prep_inputs(logits, lens, transitions)
    res = run_bass_kernel_spmd(nc, in_maps, list(range(NCORES)))
    return _postprocess(res.results, lens, transitions)


# revision 12
# speedup vs baseline: 2.4785x; 1.0023x over previous
"""CRF forward (log partition) on 8 NeuronCores, data-parallel over batch.

Math: the forward recurrence runs in probability space: with E = exp(T) and
G_t = exp(emissions_t), alpha_{t+1} = logit_t + LSE_j(T + alpha_t) becomes the
linear recurrence P_{t+1} = G_t o (E @ P_t).

All normalization is folded into the DATA on the host: each active step's
emission row is pre-scaled by 1/m_t[b] with m_t[b] = sum_i G[b,t,i]*rowmean(E)_i
(a deterministic per-sequence scalar), which keeps the state O(1) in bf16 range
without any data-dependent renorm on device.  The log-scales are accumulated in
float64 host-side and added back at the end.

Variable lengths via an extra DONE label D per group (46 labels on device):
E'[D,:45] = E[STOP,:], E'[D,D] = 1.0 (exact in bf16), column D otherwise 0.
Active steps emit 0 for D so P[D] stays exactly 0; the absorb step at t=len[b]
emits onehot(D), capturing LSE_j(T[STOP,j]+alpha_j) -- the final answer -- into
P[D]; later steps emit onehot(D) again, multiplying P[D] by exactly 1.0.

Shrinking-width steps: sequences are dealt longest-first round-robin across the
16 (core, group) slots, and within each slot sorted descending into columns, so
column k's sequence dies no later than a STATIC schedule width n_t allows.  The
state lives in ONE in-place tile; step t only updates columns [0, n_t), so dead
columns keep their DONE value frozen.  Any sequence too long for its column
(impossible under the static margin for uniform lengths, but checked) is
computed exactly on the host instead.

Per-core critical path per step: one bf16 matmul [92,92]x[92,n_t] with the
stationary blockdiag(E'^T,E'^T) kept loaded in the PE array (standalone
ldweights + non-self-loading matmuls), then one DVE tensor_mul.
"""

import numpy as np
import ml_dtypes

import concourse.bacc as bacc
import concourse.bass as bass
import concourse.mybir as mybir
import concourse.tile as tile
from concourse.bass_utils import run_bass_kernel_spmd

L = 45
START = 43
STOP = 44
LD = 46                    # labels + DONE landing pad
DONE = 45
B = 1024
S = 512
NCORES = 8
BPC = B // NCORES          # 128 sequences per core
NG = 2                     # groups per core
WCOL = BPC // NG           # 64 columns per group
PR = NG * LD               # 92 partition rows for packed state
TSTEPS = S + 1             # +1 appended absorb step
NSLOTS = NCORES * NG       # 16 (core, group) slots

F32 = mybir.dt.float32
BF16 = mybir.dt.bfloat16
NP_BF16 = ml_dtypes.bfloat16

# Static shrinking-width schedule: step t in [1, 512] updates columns [0, n_t).
# n_1 == WCOL always, so the in-place state is fully written by step 1.
_T_ARR = np.arange(1, TSTEPS)
_N_SCHED = np.minimum(
    WCOL, np.maximum(4, np.ceil(WCOL * (TSTEPS - _T_ARR) / TSTEPS).astype(int) + 2)
)
assert _N_SCHED[0] == WCOL
# Column lifetime: last step that still updates column k.
_T_COL = np.array(
    [int((np.where(_N_SCHED > k)[0] + 1).max()) for k in range(WCOL)], np.int64
)
# Per-step g block widths (block 0 is the full-width init state) and offsets.
_BLK_W = np.concatenate([[WCOL], _N_SCHED])          # [TSTEPS]
_BLK_OFF = np.concatenate([[0], np.cumsum(_BLK_W)])  # [TSTEPS+1]
GCOLS = int(_BLK_OFF[-1])
# Chunk boundaries (step indices): tiny leading chunks for a fast start.
_CHUNK_STEPS = [0, 1, 9, 41] + list(np.linspace(41, TSTEPS, 7).astype(int)[1:])
NCHUNK = len(_CHUNK_STEPS) - 1


def _build_nc():
    # Bacc (not raw Bass): its legalization splits multi-sem waits into
    # standalone event-semaphore instructions, which walrus codegen requires.
    nc = bacc.Bacc("TRN2", target_bir_lowering=False, debug=False, num_devices=NCORES)
    g_dram = nc.dram_tensor("g", [PR, GCOLS], BF16, kind="ExternalInput")
    e2t_dram = nc.dram_tensor("e2t", [PR, PR], BF16, kind="ExternalInput")
    wout_dram = nc.dram_tensor("wout", [PR, WCOL], BF16, kind="ExternalOutput")

    with tile.TileContext(nc) as tc:
        with (
            tc.tile_pool(name="const", bufs=1) as const_pool,
            tc.tile_pool(name="gchunks", bufs=1) as g_pool,
            tc.tile_pool(name="state", bufs=1) as state_pool,
            tc.tile_pool(name="ps_s", bufs=3, space="PSUM") as ps_s,
        ):
            e2t = const_pool.tile([PR, PR], BF16, tag="e2t")
            nc.sync.dma_start(e2t[:], e2t_dram[:])

            # First three chunks go out on separate engine queues so their
            # DGE setups overlap (the chain starts as soon as chunk 0+1 land).
            dma_eng = [nc.gpsimd, nc.scalar] + [nc.sync] * NCHUNK
            gtiles = []
            for c in range(NCHUNK):
                c0 = int(_BLK_OFF[_CHUNK_STEPS[c]])
                c1 = int(_BLK_OFF[_CHUNK_STEPS[c + 1]])
                gt = g_pool.tile([PR, c1 - c0], BF16, tag=f"g{c}")
                dma_eng[c].dma_start(gt[:], g_dram[:, c0:c1])
                gtiles.append(gt)

            # Load blockdiag(E'^T, E'^T) into the PE array once; every step
            # matmul below reuses it (redundant auto-ldweights are stripped
            # after tile legalization below).
            nc.tensor.ldweights(e2t[:])

            # In-place state: step 1 is full width, so the state tile is
            # fully written by the first tensor_mul; the first matmul's
            # moving operand is the host-folded W_0 block of g directly.
            w_state = state_pool.tile([PR, WCOL], BF16, tag="w")

            chunk_of = np.searchsorted(_CHUNK_STEPS, np.arange(TSTEPS), "right") - 1
            for t in range(1, TSTEPS):
                n = int(_N_SCHED[t - 1])
                moving = gtiles[0][:, 0:WCOL] if t == 1 else w_state[:, 0:n]
                s_ps = ps_s.tile([PR, WCOL], F32, tag="s")
                nc.tensor.matmul(s_ps[:, 0:n], e2t[:], moving, start=True, stop=True)
                c = int(chunk_of[t])
                off = int(_BLK_OFF[t] - _BLK_OFF[_CHUNK_STEPS[c]])
                nc.vector.tensor_mul(
                    w_state[:, 0:n], gtiles[c][:, off : off + n], s_ps[:, 0:n]
                )

            nc.sync.dma_start(wout_dram[:], w_state[:])

    # Tile legalization splits every bf16 matmult into LDWEIGHTS + MATMULT.
    # All those loads are of the SAME stationary tile, so keep only the
    # first (the explicit one above) and drop the rest.  The auto-inserted
    # loads carry no semaphore waits/updates (all sync lives on the
    # matmults), so removal is sync-neutral.
    kept_first = False
    for blk in nc.main_func.blocks:
        for i in list(blk.instructions):
            if isinstance(i, mybir.InstLdweights):
                if not kept_first:
                    kept_first = True
                elif i.sync_info is None:
                    blk.instructions.remove(i)

    nc.compile()
    return nc


_NC_CACHE = {}


def _get_nc():
    if "nc" not in _NC_CACHE:
        _NC_CACHE["nc"] = _build_nc()
    return _NC_CACHE["nc"]


def _host_norm(logit_b, len_b, T):
    """Exact float64 log-space forward for one sequence (fallback path)."""
    NEG_INF = -10000.0
    alpha = np.full(L, NEG_INF)
    alpha[START] = 0.0
    for t in range(len_b):
        mat = T + alpha[None, :]
        mx = mat.max(axis=1)
        alpha = logit_b[t] + np.log(np.exp(mat - mx[:, None]).sum(axis=1)) + mx
    v = alpha + T[STOP]
    mx = v.max()
    return np.log(np.exp(v - mx).sum()) + mx


def _prep_inputs(logits, lens, transitions):
    """Host-side preprocessing: exp + absorb-rewrite + deterministic
    per-(seq,step) scaling + length-sorted packing.  Stashes the float64
    log-scale accumulator, the column permutation, and any host-fallback
    results for _postprocess."""
    logits = np.asarray(logits, np.float32)
    lens = np.asarray(lens, np.int64)
    T = np.asarray(transitions, np.float64)

    E = np.exp(T)                      # [45,45] float64
    erow = E.mean(axis=1)              # mean_j E[i,j], [45]

    Eg = np.zeros((LD, LD), np.float64)
    Eg[:L, :L] = E
    Eg[DONE, :L] = E[STOP, :]
    Eg[DONE, DONE] = 1.0
    e2t = np.zeros((PR, PR), np.float64)
    e2t[:LD, :LD] = Eg.T
    e2t[LD:, LD:] = Eg.T

    G = np.exp(logits.astype(np.float64))          # [B,S,45]

    t_idx = np.arange(S)[None, :]                  # [1,S]
    active = t_idx < lens[:, None]                 # [B,S]

    # Fold step 0 and normalize it exactly: W0 = G0*E[:,START], scale 1/sum.
    W0 = G[:, 0, :] * E[:, START][None, :]         # [B,45]
    m0 = W0.sum(axis=1)                            # [B]
    G[:, 0, :] = W0 / m0[:, None]

    # Active steps t>=1: scale by 1/m_t, m_t = sum_i G_t[i]*erow[i].
    m = G @ erow                                   # [B,S]
    scale_mask = active & (t_idx > 0)
    np.divide(G, m[:, :, None], out=G, where=scale_mask[:, :, None])

    # log-scale accumulator: z[b] = log m0 + sum_{1<=t<len} log m_t.
    logm = np.where(scale_mask, np.log(m), 0.0)
    z = np.log(m0) + logm.sum(axis=1)

    # 46-label emissions: D gets 0 while active, onehot(D) from t>=len on.
    G46 = np.zeros((B, TSTEPS, LD), np.float64)
    G46[:, :S, :L] = np.where(active[:, :, None], G, 0.0)
    done_from = t_idx >= lens[:, None]             # includes absorb step
    G46[:, :S, DONE] = np.where(done_from, 1.0, 0.0)
    G46[:, S, DONE] = 1.0                          # appended step

    # Deal longest-first round-robin across the 16 (core, group) slots.
    order = np.argsort(-lens, kind="stable")
    slots = np.empty((NSLOTS, WCOL), np.int64)
    for r, b in enumerate(order):
        slots[r % NSLOTS][r // NSLOTS] = b
    # Host fallback for any sequence outliving its column's static lifetime.
    host_norms = {}
    logits64 = logits.astype(np.float64)
    for s in range(NSLOTS):
        for k in range(WCOL):
            b = slots[s][k]
            if lens[b] > _T_COL[k]:
                host_norms[int(b)] = _host_norm(logits64[b], int(lens[b]), T)

    _NC_CACHE["z"] = z
    _NC_CACHE["slots"] = slots
    _NC_CACHE["host_norms"] = host_norms

    g16 = G46.astype(NP_BF16)
    e2t16 = e2t.astype(NP_BF16)
    in_maps = []
    for c in range(NCORES):
        g_in = np.zeros((PR, GCOLS), NP_BF16)
        for g in range(NG):
            seqs = slots[c * NG + g]               # [WCOL] original indices
            rows = slice(g * LD, (g + 1) * LD)
            # Per-step blocks: step t occupies cols [_BLK_OFF[t], +width).
            gc = g16[seqs]                         # [WCOL, TSTEPS, LD]
            for t in range(TSTEPS):
                w = int(_BLK_W[t])
                o = int(_BLK_OFF[t])
                g_in[rows, o : o + w] = gc[:w, t, :].T
        in_maps.append({"g": g_in, "e2t": e2t16})
    return in_maps


def _postprocess(results, lens, transitions):
    z = _NC_CACHE["z"]
    slots = _NC_CACHE["slots"]
    host_norms = _NC_CACHE["host_norms"]
    norm = np.empty(B, np.float64)
    for c in range(NCORES):
        wout = np.asarray(results[c]["wout"]).astype(np.float64)  # [PR, WCOL]
        for g in range(NG):
            seqs = slots[c * NG + g]
            pdone = wout[g * LD + DONE, :]
            norm[seqs] = np.log(pdone) + z[seqs]
    for b, v in host_norms.items():
        norm[b] = v
    return norm.astype(np.float32)


def kernel(logits, lens, transitions):
    nc = _get_nc()
    in_maps = _prep_inputs(logits, lens, transitions)
    res = run_bass_kernel_spmd(nc, in_maps, list(range(NCORES)))
    return _postprocess(res.results, lens, transitions)
